# revision 3
# baseline (speedup 1.0000x reference)
"""Trainium2 Bass kernel for nn_DSRB_19447611916345 (dense_cnn).

Reference math (per batch image, C=256, H=W=128):
    S    = 0.25*(conv1x1_s1(x) + ... + conv1x1_s4(x))   four (+-2,+-2)-shifted 1x1 convs
    res  = 2*sigmoid(x - S) - 1 = tanh(0.5*(x - S))
    h    = relu(x * res)
    y    = mean_{H,W}(h)                                 AGCA channel attention
    y1   = agca_w1 @ y;  a1 = sigmoid(w2*y1)
    y2   = y1*a1 + A2.T @ y1;  y3 = relu(w3*y2)
    gate = sigmoid(agca_w4 @ y3)
    out  = h * gate

Sharding: data-parallel over batch B=8 across 8 NeuronCores (weights
replicated, no collectives). On-device per core:
  - shifted convs via 8 accumulating bf16 matmuls per [128,512] output tile
    (4 shifts x 2 input-channel halves), PSUM f32 accumulation
  - elementwise: DVE subtract, ACT tanh, GPSIMD multiply, ACT relu with
    accum_out (fused spatial-mean partial sums)
  - AGCA tail entirely in f32 (tiny matvecs on the PE)
  - phase 2: per-channel gate multiply + store
Host prep: weight transpose/scale (lhsT layout, 0.25 factor folded in,
cast to bf16) and a zero-padded bf16 copy of x so shifted matmul operands
are plain DMA loads (borders come in pre-zeroed).
"""

import numpy as np
import ml_dtypes

import concourse.bacc as bacc
import concourse.mybir as mybir
import concourse.tile as tile
from concourse.bass_utils import run_bass_kernel_spmd

f32 = mybir.dt.float32
bf16 = mybir.dt.bfloat16
Alu = mybir.AluOpType
Act = mybir.ActivationFunctionType

B = 8
C = 256
H = 128
W = 128
HD = 64            # AGCA hidden dim
P = 128            # SBUF partitions
KH = C // P        # 2 input-channel halves
MH = C // P        # 2 output-channel halves
RB = 4             # rows per block
NBLK = H // RB     # 32
NT = RB * W        # 512, matmul free dim / PSUM bank
PADW = W + 4       # 132
SHIFTS = [(0, 0), (4, 0), (0, 4), (4, 4)]
PH2_CHUNK = 4      # phase-2 blocks per DMA

_STATE = {}


def _build():
    nc = bacc.Bacc(name="dsrb")
    xf_d = nc.dram_tensor("xf", [P, KH, H, W], f32, kind="ExternalInput")
    xp_d = nc.dram_tensor("xp", [P, H + 4, KH, PADW], bf16, kind="ExternalInput")
    wl_d = nc.dram_tensor("wl", [P, len(SHIFTS), KH, MH, P], bf16, kind="ExternalInput")
    bneg_d = nc.dram_tensor("bneg", [P, MH], f32, kind="ExternalInput")
    aw1_d = nc.dram_tensor("aw1", [P, KH, HD], f32, kind="ExternalInput")
    a2_d = nc.dram_tensor("a2", [HD, HD], f32, kind="ExternalInput")
    aw4_d = nc.dram_tensor("aw4", [HD, MH, P], f32, kind="ExternalInput")
    sc_d = nc.dram_tensor("sc", [P, 2], f32, kind="ExternalInput")
    out_d = nc.dram_tensor("out", [P, MH, H, W], f32, kind="ExternalOutput")

    with tile.TileContext(nc) as tc:
        with (
            tc.tile_pool(name="const", bufs=1) as constp,
            tc.tile_pool(name="xin", bufs=4) as xinp,
            tc.tile_pool(name="grp", bufs=6) as grpp,
            tc.tile_pool(name="big", bufs=1) as bigp,
            tc.tile_pool(name="mm", bufs=4) as mmp,
            tc.tile_pool(name="agca", bufs=1) as agp,
            tc.tile_pool(name="ps", bufs=4, space="PSUM") as psp,
            tc.tile_pool(name="psag", bufs=1, space="PSUM") as psagp,
        ):
            wt = constp.tile([P, len(SHIFTS), KH, MH, P], bf16)
            nc.sync.dma_start(out=wt, in_=wl_d[:])
            bneg = constp.tile([P, MH], f32)
            nc.sync.dma_start(out=bneg, in_=bneg_d[:, :])
            aw1 = constp.tile([P, KH, HD], f32)
            nc.sync.dma_start(out=aw1, in_=aw1_d[:])
            a2t = constp.tile([HD, HD], f32)
            nc.sync.dma_start(out=a2t, in_=a2_d[:, :])
            aw4 = constp.tile([HD, MH, P], f32)
            nc.sync.dma_start(out=aw4, in_=aw4_d[:])
            sct = constp.tile([P, 2], f32)
            nc.sync.dma_start(out=sct, in_=sc_d[:, :])

            hres = bigp.tile([P, MH, H * W], f32)
            partials = bigp.tile([P, MH, NBLK], f32)

            xts, grps = {}, {}

            def load_group(g):
                t = grpp.tile([P, RB, KH, PADW], bf16, tag="grp")
                nc.sync.dma_start(out=t, in_=xp_d[:, 4 * g : 4 * g + 4, :, :])
                grps[g] = t

            def compute_block(j):
                for mh in range(MH):
                    ps = psp.tile([P, NT], f32)
                    nmm = len(SHIFTS) * KH
                    i = 0
                    for si, (dr, dw) in enumerate(SHIFTS):
                        g = j + dr // RB
                        for kh in range(KH):
                            nc.tensor.matmul(
                                ps,
                                wt[:, si, kh, mh, :],
                                grps[g][:, :, kh, dw : dw + W],
                                start=(i == 0),
                                stop=(i == nmm - 1),
                            )
                            i += 1
                    xtj = xts[j][:, mh].rearrange("p a b -> p (a b)")
                    uu = mmp.tile([P, NT], f32, tag="u")
                    nc.vector.tensor_tensor(out=uu, in0=xtj, in1=ps, op=Alu.subtract)
                    res_t = mmp.tile([P, NT], f32, tag="res")
                    nc.scalar.activation(
                        out=res_t, in_=uu, func=Act.Tanh,
                        bias=bneg[:, mh : mh + 1], scale=0.5,
                    )
                    hp_t = mmp.tile([P, NT], f32, tag="hp")
                    nc.gpsimd.tensor_tensor(out=hp_t, in0=xtj, in1=res_t, op=Alu.mult)
                    nc.scalar.activation(
                        out=hres[:, mh, NT * j : NT * (j + 1)],
                        in_=hp_t,
                        func=Act.Relu,
                        accum_out=partials[:, mh, j : j + 1],
                    )

            load_group(0)
            for j in range(NBLK):
                load_group(j + 1)
                xt = xinp.tile([P, MH, RB, W], f32, tag="xt")
                nc.sync.dma_start(out=xt, in_=xf_d[:, :, 4 * j : 4 * j + 4, :])
                xts[j] = xt
                compute_block(j)
                xts.pop(j - 1, None)
                grps.pop(j - 1, None)

            # ---- AGCA tail (all f32) ----
            ysum = agp.tile([P, KH], f32)
            for kh in range(KH):
                nc.vector.tensor_reduce(
                    out=ysum[:, kh : kh + 1],
                    in_=partials[:, kh, :],
                    axis=mybir.AxisListType.X,
                    op=Alu.add,
                )
            y1ps = psagp.tile([HD, 1], f32)
            for kh in range(KH):
                nc.tensor.matmul(
                    y1ps, aw1[:, kh, :], ysum[:, kh : kh + 1],
                    start=(kh == 0), stop=(kh == KH - 1),
                )
            y1 = agp.tile([HD, 1], f32)
            nc.vector.tensor_copy(out=y1, in_=y1ps)
            a1 = agp.tile([HD, 1], f32)
            nc.scalar.activation(
                out=a1, in_=y1ps, func=Act.Sigmoid, scale=sct[:HD, 0:1]
            )
            y2ps = psagp.tile([HD, 1], f32)
            nc.tensor.matmul(y2ps, a2t[:, :], y1, start=True, stop=True)
            y2 = agp.tile([HD, 1], f32)
            nc.vector.scalar_tensor_tensor(
                out=y2, in0=y1, scalar=a1, in1=y2ps, op0=Alu.mult, op1=Alu.add
            )
            y3 = agp.tile([HD, 1], f32)
            nc.scalar.activation(
                out=y3, in_=y2, func=Act.Relu, scale=sct[:HD, 1:2]
            )
            gate = agp.tile([P, MH], f32)
            for mh in range(MH):
                gps = psagp.tile([P, 1], f32)
                nc.tensor.matmul(gps, aw4[:, mh, :], y3, start=True, stop=True)
                nc.scalar.activation(
                    out=gate[:, mh : mh + 1], in_=gps, func=Act.Sigmoid
                )

            # ---- phase 2: out = h * gate ----
            CL = PH2_CHUNK * NT
            for c0 in range(0, NBLK, PH2_CHUNK):
                lo = c0 * NT
                for mh in range(MH):
                    blk = hres[:, mh, lo : lo + CL]
                    nc.vector.tensor_scalar_mul(
                        out=blk, in0=blk, scalar1=gate[:, mh : mh + 1]
                    )
                nc.sync.dma_start(
                    out=out_d[:, :, RB * c0 : RB * (c0 + PH2_CHUNK), :].rearrange(
                        "p k a b -> p k (a b)"
                    ),
                    in_=hres[:, :, lo : lo + CL],
                )

    nc.finalize()
    return nc


def _prep_core_inputs(xb, shared):
    """xb: [C, H, W] f32 for one batch image."""
    x4 = xb.reshape(KH, P, H, W)
    xf = np.ascontiguousarray(x4.transpose(1, 0, 2, 3))
    xp = np.zeros((P, H + 4, KH, PADW), ml_dtypes.bfloat16)
    xp[:, 2 : H + 2, :, 2 : W + 2] = x4.transpose(1, 2, 0, 3).astype(
        ml_dtypes.bfloat16
    )
    return {"xf": xf, "xp": xp, **shared}


def _prep_shared(w1, b1, w2, b2, w3, b3, w4, b4,
                 agca_w1, agca_w2, agca_w3, agca_A2, agca_w4):
    ws = np.stack([np.asarray(w) for w in (w1, w2, w3, w4)]).astype(np.float64)
    # wl[p, s, kh, mh, m] = 0.25 * w_s[mh*P+m, kh*P+p]
    wl = (0.25 * ws).reshape(len(SHIFTS), MH, P, KH, P).transpose(4, 0, 3, 1, 2)
    wl = np.ascontiguousarray(wl).astype(ml_dtypes.bfloat16)
    bsum = 0.25 * (np.asarray(b1) + np.asarray(b2) + np.asarray(b3) + np.asarray(b4))
    bneg = np.ascontiguousarray((-0.5 * bsum).reshape(MH, P).T).astype(np.float32)
    # aw1[p, kh, m] = agca_w1[m, kh*P+p] / (H*W)
    aw1 = np.ascontiguousarray(
        (np.asarray(agca_w1, np.float64) / (H * W)).reshape(HD, KH, P).transpose(2, 1, 0)
    ).astype(np.float32)
    a2 = np.ascontiguousarray(np.asarray(agca_A2, np.float32))
    # aw4[k, mh, m] = agca_w4[mh*P+m, k]
    aw4 = np.ascontiguousarray(
        np.asarray(agca_w4, np.float32).reshape(MH, P, HD).transpose(2, 0, 1)
    ).astype(np.float32)
    sc = np.broadcast_to(
        np.array([float(np.asarray(agca_w2)[0]), float(np.asarray(agca_w3)[0])],
                 np.float32),
        (P, 2),
    ).copy()
    return {"wl": wl, "bneg": bneg, "aw1": aw1, "a2": a2, "aw4": aw4, "sc": sc}


def _run(inputs, trace=False):
    if "nc" not in _STATE:
        _STATE["nc"] = _build()
    nc = _STATE["nc"]
    x = np.asarray(inputs["x"], np.float32)
    shared = _prep_shared(
        inputs["w1"], inputs["b1"], inputs["w2"], inputs["b2"],
        inputs["w3"], inputs["b3"], inputs["w4"], inputs["b4"],
        inputs["agca_w1"], inputs["agca_w2"], inputs["agca_w3"],
        inputs["agca_A2"], inputs["agca_w4"],
    )
    in_maps = [_prep_core_inputs(x[b], shared) for b in range(B)]
    r = run_bass_kernel_spmd(nc, in_maps, core_ids=list(range(B)), trace=trace)
    out = np.empty((B, C, H, W), np.float32)
    for b in range(B):
        out[b] = r.results[b]["out"].transpose(1, 0, 2, 3).reshape(C, H, W)
    return out, r


def kernel(**inputs):
    out, _ = _run(inputs, trace=False)
    return out


# revision 19
# speedup vs baseline: 65790.0739x; 65790.0739x over previous
"""Trainium2 Bass kernel for nn_DSRB_19447611916345 (dense_cnn).

Reference math (per batch image, C=256, H=W=128):
    S    = 0.25*(conv1x1_s1(x) + ... + conv1x1_s4(x))   four (+-2,+-2)-shifted 1x1 convs
    res  = 2*sigmoid(x - S) - 1 = tanh(0.5*(x - S))
    h    = relu(x * res)
    y    = mean_{H,W}(h)                                 AGCA channel attention
    y1   = agca_w1 @ y;  a1 = sigmoid(w2*y1)
    y2   = y1*a1 + A2.T @ y1;  y3 = relu(w3*y2)
    gate = sigmoid(agca_w4 @ y3)
    out  = h * gate

Sharding: data-parallel over batch B=8 across 8 NeuronCores (weights
replicated, no collectives). On-device per core:
  - shifted convs via 8 accumulating bf16 matmuls per [128,512] output tile
    (4 shifts x 2 input-channel halves), PSUM f32 accumulation
  - elementwise: DVE subtract, ACT tanh, GPSIMD multiply, ACT relu with
    accum_out (fused spatial-mean partial sums)
  - AGCA tail entirely in f32 (tiny matvecs on the PE)
  - phase 2: per-channel gate multiply + store
Host prep: weight transpose/scale (lhsT layout, 0.25 factor folded in,
cast to bf16) and a zero-padded bf16 copy of x so shifted matmul operands
are plain DMA loads (borders come in pre-zeroed).
"""

import numpy as np
import ml_dtypes

import concourse.bacc as bacc
import concourse.mybir as mybir
import concourse.tile as tile

f32 = mybir.dt.float32
bf16 = mybir.dt.bfloat16
Alu = mybir.AluOpType
Act = mybir.ActivationFunctionType

B = 8
C = 256
H = 128
W = 128
HD = 64            # AGCA hidden dim
P = 128            # SBUF partitions
KH = C // P        # 2 input-channel halves
MH = C // P        # 2 output-channel halves
RB = 4             # rows per block
NBLK = H // RB     # 32
NT = RB * W        # 512, matmul free dim / PSUM bank
PADW = W + 4       # 132
SHIFTS = [(0, 0), (4, 0), (0, 4), (4, 4)]
PH2_CHUNK = 2      # phase-2 blocks per DMA

_STATE = {}


def _build():
    nc = bacc.Bacc(name="dsrb")
    xf_d = nc.dram_tensor("xf", [P, KH, H, W], f32, kind="ExternalInput")
    xp_d = nc.dram_tensor("xp", [P, H + 4, KH, PADW], bf16, kind="ExternalInput")
    wl_d = nc.dram_tensor("wl", [P, len(SHIFTS), KH, MH, P], bf16, kind="ExternalInput")
    bneg_d = nc.dram_tensor("bneg", [P, MH], f32, kind="ExternalInput")
    aw1_d = nc.dram_tensor("aw1", [P, KH, HD], f32, kind="ExternalInput")
    a2_d = nc.dram_tensor("a2", [HD, HD], f32, kind="ExternalInput")
    aw4_d = nc.dram_tensor("aw4", [HD, MH, P], f32, kind="ExternalInput")
    sc_d = nc.dram_tensor("sc", [P, 4], f32, kind="ExternalInput")
    out_d = nc.dram_tensor("out", [P, MH, H, W], f32, kind="ExternalOutput")

    with tile.TileContext(nc) as tc:
        with (
            tc.tile_pool(name="const", bufs=1) as constp,
            tc.tile_pool(name="xin", bufs=4) as xinp,
            tc.tile_pool(name="grp", bufs=7) as grpp,
            tc.tile_pool(name="big", bufs=1) as bigp,
            tc.tile_pool(name="mm", bufs=4) as mmp,
            tc.tile_pool(name="agca", bufs=1) as agp,
            tc.tile_pool(name="ps", bufs=5, space="PSUM") as psp,
            tc.tile_pool(name="psag", bufs=1, space="PSUM") as psagp,
        ):
            wt = constp.tile([P, len(SHIFTS), KH, MH, P], bf16)
            nc.sync.dma_start(out=wt, in_=wl_d[:])
            bneg = constp.tile([P, MH], f32)
            nc.sync.dma_start(out=bneg, in_=bneg_d[:, :])
            aw1 = constp.tile([P, KH, HD], f32)
            nc.sync.dma_start(out=aw1, in_=aw1_d[:])
            a2t = constp.tile([HD, HD], f32)
            nc.sync.dma_start(out=a2t, in_=a2_d[:, :])
            aw4 = constp.tile([HD, MH, P], f32)
            nc.sync.dma_start(out=aw4, in_=aw4_d[:])
            sct = constp.tile([P, 4], f32)
            nc.sync.dma_start(out=sct, in_=sc_d[:, :])

            hres = bigp.tile([P, MH, H * W], f32)
            partials = bigp.tile([P, MH, NBLK], f32)

            xts, grps = {}, {}

            def load_group(g):
                t = grpp.tile([P, RB, KH, PADW], bf16, tag="grp")
                nc.sync.dma_start(out=t, in_=xp_d[:, 4 * g : 4 * g + 4, :, :])
                grps[g] = t

            load_group(0)
            load_group(1)

            def compute_block(j):
                for mh in range(MH):
                    ps = psp.tile([P, NT], f32)
                    nmm = len(SHIFTS) * KH
                    i = 0
                    for si, (dr, dw) in enumerate(SHIFTS):
                        g = j + dr // RB
                        for kh in range(KH):
                            nc.tensor.matmul(
                                ps,
                                wt[:, si, kh, mh, :],
                                grps[g][:, :, kh, dw : dw + W],
                                start=(i == 0),
                                stop=(i == nmm - 1),
                            )
                            i += 1
                    xtj = xts[j][:, mh].rearrange("p a b -> p (a b)")
                    uu = mmp.tile([P, NT], f32, tag="u")
                    nc.vector.tensor_tensor(out=uu, in0=xtj, in1=ps, op=Alu.subtract)
                    res_t = mmp.tile([P, NT], f32, tag="res")
                    nc.scalar.activation(
                        out=res_t, in_=uu, func=Act.Tanh,
                        bias=bneg[:, mh : mh + 1], scale=0.5,
                    )
                    hp_t = mmp.tile([P, NT], f32, tag="hp")
                    nc.gpsimd.tensor_tensor(out=hp_t, in0=xtj, in1=res_t, op=Alu.mult)
                    nc.vector.tensor_scalar(
                        out=hres[:, mh, NT * j : NT * (j + 1)],
                        in0=hp_t,
                        scalar1=0.0,
                        scalar2=0.0,
                        op0=Alu.max,
                        op1=Alu.add,
                        accum_out=partials[:, mh, j : j + 1],
                    )

            for j in range(NBLK):
                if 2 <= j + 2 <= NBLK:
                    load_group(j + 2)
                xt = xinp.tile([P, MH, RB, W], f32, tag="xt")
                nc.sync.dma_start(out=xt, in_=xf_d[:, :, 4 * j : 4 * j + 4, :])
                xts[j] = xt
                compute_block(j)
                xts.pop(j - 1, None)
                grps.pop(j - 1, None)

            # ---- AGCA tail (all f32) ----
            ysum = agp.tile([P, KH], f32)
            for kh in range(KH):
                nc.vector.tensor_reduce(
                    out=ysum[:, kh : kh + 1],
                    in_=partials[:, kh, :],
                    axis=mybir.AxisListType.X,
                    op=Alu.add,
                )
            y1ps = psagp.tile([HD, 1], f32)
            for kh in range(KH):
                nc.tensor.matmul(
                    y1ps, aw1[:, kh, :], ysum[:, kh : kh + 1],
                    start=(kh == 0), stop=(kh == KH - 1),
                )
            y1 = agp.tile([HD, 1], f32)
            nc.vector.tensor_copy(out=y1, in_=y1ps)
            a1 = agp.tile([HD, 1], f32)
            nc.scalar.activation(
                out=a1, in_=y1ps, func=Act.Tanh, scale=sct[:HD, 2:3]
            )
            nc.vector.tensor_scalar(
                out=a1, in0=a1, scalar1=0.5, scalar2=0.5,
                op0=Alu.mult, op1=Alu.add,
            )
            y2ps = psagp.tile([HD, 1], f32)
            nc.tensor.matmul(y2ps, a2t[:, :], y1, start=True, stop=True)
            y2 = agp.tile([HD, 1], f32)
            nc.vector.scalar_tensor_tensor(
                out=y2, in0=y1, scalar=a1, in1=y2ps, op0=Alu.mult, op1=Alu.add
            )
            y3 = agp.tile([HD, 1], f32)
            nc.scalar.activation(
                out=y3, in_=y2, func=Act.Relu, scale=sct[:HD, 1:2]
            )
            gate = agp.tile([P, MH], f32)
            for mh in range(MH):
                gps = psagp.tile([P, 1], f32)
                nc.tensor.matmul(gps, aw4[:, mh, :], y3, start=True, stop=True)
                nc.scalar.activation(
                    out=gate[:, mh : mh + 1], in_=gps, func=Act.Tanh, scale=0.5
                )
            nc.vector.tensor_scalar(
                out=gate, in0=gate, scalar1=0.5, scalar2=0.5,
                op0=Alu.mult, op1=Alu.add,
            )

            # ---- phase 2: out = h * gate ----
            CL = PH2_CHUNK * NT
            for ci, c0 in enumerate(range(0, NBLK, PH2_CHUNK)):
                lo = c0 * NT
                for mh in range(MH):
                    blk = hres[:, mh, lo : lo + CL]
                    if (ci + mh) % 2 == 0:
                        nc.vector.tensor_scalar_mul(
                            out=blk, in0=blk, scalar1=gate[:, mh : mh + 1]
                        )
                    else:
                        nc.scalar.mul(out=blk, in_=blk,
                                      mul=gate[:, mh : mh + 1])
                nc.sync.dma_start(
                    out=out_d[:, :, RB * c0 : RB * (c0 + PH2_CHUNK), :].rearrange(
                        "p k a b -> p k (a b)"
                    ),
                    in_=hres[:, :, lo : lo + CL],
                )

    nc.finalize()
    return nc


def _prep_core_inputs(xb, shared):
    """xb: [C, H, W] f32 for one batch image."""
    x4 = xb.reshape(KH, P, H, W)
    xf = np.ascontiguousarray(x4.transpose(1, 0, 2, 3))
    xp = np.zeros((P, H + 4, KH, PADW), ml_dtypes.bfloat16)
    xp[:, 2 : H + 2, :, 2 : W + 2] = x4.transpose(1, 2, 0, 3).astype(
        ml_dtypes.bfloat16
    )
    return {"xf": xf, "xp": xp, **shared}


def _prep_shared(w1, b1, w2, b2, w3, b3, w4, b4,
                 agca_w1, agca_w2, agca_w3, agca_A2, agca_w4):
    ws = np.stack([np.asarray(w) for w in (w1, w2, w3, w4)]).astype(np.float64)
    # wl[p, s, kh, mh, m] = 0.25 * w_s[mh*P+m, kh*P+p]
    wl = (0.25 * ws).reshape(len(SHIFTS), MH, P, KH, P).transpose(4, 0, 3, 1, 2)
    wl = np.ascontiguousarray(wl).astype(ml_dtypes.bfloat16)
    bsum = 0.25 * (np.asarray(b1) + np.asarray(b2) + np.asarray(b3) + np.asarray(b4))
    bneg = np.ascontiguousarray((-0.5 * bsum).reshape(MH, P).T).astype(np.float32)
    # aw1[p, kh, m] = agca_w1[m, kh*P+p] / (H*W)
    aw1 = np.ascontiguousarray(
        (np.asarray(agca_w1, np.float64) / (H * W)).reshape(HD, KH, P).transpose(2, 1, 0)
    ).astype(np.float32)
    a2 = np.ascontiguousarray(np.asarray(agca_A2, np.float32))
    # aw4[k, mh, m] = agca_w4[mh*P+m, k]
    aw4 = np.ascontiguousarray(
        np.asarray(agca_w4, np.float32).reshape(MH, P, HD).transpose(2, 0, 1)
    ).astype(np.float32)
    w2v = float(np.asarray(agca_w2)[0])
    w3v = float(np.asarray(agca_w3)[0])
    sc = np.broadcast_to(
        np.array([w2v, w3v, 0.5 * w2v, 0.0], np.float32), (P, 4)
    ).copy()
    return {"wl": wl, "bneg": bneg, "aw1": aw1, "a2": a2, "aw4": aw4, "sc": sc}


def _get_runner(nc):
    """Cached shard_map-jitted executor mirroring bass2jax.run_bass_via_pjrt's
    multi-core path, so repeat kernel() calls don't re-trace/re-jit."""
    import jax
    import concourse.mybir as mb
    from concourse import bass2jax
    from jax.sharding import Mesh, PartitionSpec
    from jax.experimental.shard_map import shard_map

    bass2jax.install_neuronx_cc_hook()
    partition_name = (
        nc.partition_id_tensor.name if nc.partition_id_tensor else None
    )
    in_names, out_names, out_avals, zero_shapes = [], [], [], []
    for alloc in nc.m.functions[0].allocations:
        if not isinstance(alloc, mb.MemoryLocationSet):
            continue
        name = alloc.memorylocations[0].name
        if alloc.kind == "ExternalInput":
            if name != partition_name:
                in_names.append(name)
        elif alloc.kind == "ExternalOutput":
            out_names.append(name)
            shape = tuple(alloc.tensor_shape)
            dtype = mb.dt.np(alloc.dtype)
            out_avals.append(jax.core.ShapedArray(shape, dtype))
            zero_shapes.append((shape, dtype))
    n_params = len(in_names)
    n_outs = len(out_avals)
    all_in_names = list(in_names) + list(out_names)
    if partition_name is not None:
        all_in_names.append(partition_name)
    donate = tuple(range(n_params, n_params + n_outs))

    def _body(*args):
        operands = list(args)
        if partition_name is not None:
            operands.append(bass2jax.partition_id_tensor())
        outs = bass2jax._bass_exec_p.bind(
            *operands,
            out_avals=tuple(out_avals),
            in_names=tuple(all_in_names),
            out_names=tuple(out_names),
            lowering_input_output_aliases=(),
            sim_require_finite=True,
            sim_require_nnan=True,
            nc=nc,
        )
        return tuple(outs)

    devices = jax.devices()[:B]
    mesh = Mesh(np.asarray(devices), ("core",))
    in_specs = (PartitionSpec("core"),) * (n_params + n_outs)
    out_specs = (PartitionSpec("core"),) * n_outs
    sharded = jax.jit(
        shard_map(_body, mesh=mesh, in_specs=in_specs, out_specs=out_specs,
                  check_rep=False),
        donate_argnums=donate,
        keep_unused=True,
    )

    def run(in_maps):
        concat_in = [
            np.concatenate([np.asarray(in_maps[c][nm]) for c in range(B)], axis=0)
            for nm in in_names
        ]
        concat_zeros = [
            np.zeros((B * s[0], *s[1:]), d) for s, d in zero_shapes
        ]
        out_arrs = sharded(*concat_in, *concat_zeros)
        return [
            {
                nm: np.asarray(out_arrs[i]).reshape(B, *out_avals[i].shape)[c]
                for i, nm in enumerate(out_names)
            }
            for c in range(B)
        ]

    return run


def _run(inputs, trace=False):
    if "nc" not in _STATE:
        _STATE["nc"] = _build()
    nc = _STATE["nc"]
    x = np.asarray(inputs["x"], np.float32)
    shared = _prep_shared(
        inputs["w1"], inputs["b1"], inputs["w2"], inputs["b2"],
        inputs["w3"], inputs["b3"], inputs["w4"], inputs["b4"],
        inputs["agca_w1"], inputs["agca_w2"], inputs["agca_w3"],
        inputs["agca_A2"], inputs["agca_w4"],
    )
    in_maps = [_prep_core_inputs(x[b], shared) for b in range(B)]
    if "runner" not in _STATE:
        _STATE["runner"] = _get_runner(nc)
    results = _STATE["runner"](in_maps)
    out = np.empty((B, C, H, W), np.float32)
    for b in range(B):
        out[b] = results[b]["out"].transpose(1, 0, 2, 3).reshape(C, H, W)
    return out, results


def kernel(**inputs):
    out, _ = _run(inputs, trace=False)
    return out
